# revision 14
# baseline (speedup 1.0000x reference)
"""Multi-head self-attention on 8 Trainium2 NeuronCores (Bass/Tile, SPMD).

Sharding (head/tensor parallel, per the row/col-sharded projection scheme):
  - 16 heads / 8 cores = 2 heads per core.
  - Each core receives x^T [1024, 4096] (full activations, transposed on
    host), the 128-column slice of wq/wk/wv (+bias slices) for its 2 heads,
    and the matching 128-row slice of wo.
  - On-core: Q^T/K^T/V^T projections (fp32r matmuls, contraction over
    d_model), V transposed to natural [seq, dk] layout via PE-transpose,
    attention computed with scores TRANSPOSED (S^T[k, q] = K_h Q_h^T) so the
    softmax denominator can be formed with an all-ones matmul that also
    broadcasts it across partitions, then a row-parallel partial of the
    output projection: out_partial^T = wo_slice^T-contract ctx^T.
  - Host unshards by summing the 8 partials and adding the output bias.

Softmax: scores/8 for this problem lie in [-2.7, 2.7], so exp() needs no
max-subtraction (a constant shift cancels exactly in softmax anyway).
"""

import numpy as np

import concourse.bass as bass  # noqa: F401  (bass types used via tile/bacc)
import concourse.mybir as mybir
import concourse.tile as tile
from concourse import bacc
from concourse.bass_utils import run_bass_kernel_spmd

B, S, D, H, DK = 2, 2048, 1024, 16, 64
R = B * S            # 4096 flattened (batch*seq) rows
NCORES = 8
PC = D // NCORES     # 128 projection columns per core (2 heads x 64)
HPC = PC // DK       # 2 heads per core

RCW = 512            # row-chunk width for the projection phase
NRC = R // RCW       # 8 row chunks
NKT = S // 128       # 16 key tiles per batch
QCW = 1024           # query-chunk width for the attention phase
NQC = S // QCW       # 2 query chunks per batch

F32 = mybir.dt.float32
F32R = mybir.dt.float32r
EXP = mybir.ActivationFunctionType.Exp

_CACHE = {}


def _build_nc():
    nc = bacc.Bacc(None, target_bir_lowering=False, debug=False)

    xt = nc.declare_dram_parameter("xt", [D, R], F32R, isOutput=False)
    wq = nc.declare_dram_parameter("wq", [D, PC], F32R, isOutput=False)
    wk = nc.declare_dram_parameter("wk", [D, PC], F32R, isOutput=False)
    wv = nc.declare_dram_parameter("wv", [D, PC], F32R, isOutput=False)
    wo = nc.declare_dram_parameter("wo", [PC, D], F32R, isOutput=False)
    bq = nc.declare_dram_parameter("bq", [PC, 1], F32, isOutput=False)
    bk = nc.declare_dram_parameter("bk", [PC, 1], F32, isOutput=False)
    bv = nc.declare_dram_parameter("bv", [PC, 1], F32, isOutput=False)
    ident = nc.declare_dram_parameter("ident", [128, DK], F32, isOutput=False)
    ones = nc.declare_dram_parameter("ones", [128, DK], F32R, isOutput=False)
    out = nc.declare_dram_parameter("out", [D, R], F32, isOutput=True)

    with tile.TileContext(nc) as tc:
        with (
            tc.tile_pool(name="const", bufs=1) as constp,
            tc.tile_pool(name="persist", bufs=1) as persist,
            tc.tile_pool(name="xin", bufs=2) as xin,
            tc.tile_pool(name="vtmp", bufs=2) as vtmp,
            tc.tile_pool(name="ptp", bufs=5) as ptp,
            tc.tile_pool(name="recp", bufs=2) as recp,
            tc.tile_pool(name="ostage", bufs=4) as ostage,
            tc.tile_pool(name="psA", bufs=2, space="PSUM") as psA,
            tc.tile_pool(name="psB", bufs=2, space="PSUM") as psB,
        ):
            # ---- constants ----
            wq_sb = constp.tile([128, D // 128, PC], F32R, tag="wq")
            wk_sb = constp.tile([128, D // 128, PC], F32R, tag="wk")
            wv_sb = constp.tile([128, D // 128, PC], F32R, tag="wv")
            wo_sb = constp.tile([128, D], F32R, tag="wo")
            bq_sb = constp.tile([128, 1], F32, tag="bq")
            bk_sb = constp.tile([128, 1], F32, tag="bk")
            bv_sb = constp.tile([128, 1], F32, tag="bv")
            id_sb = constp.tile([128, DK], F32, tag="id")
            ones_sb = constp.tile([128, DK], F32R, tag="ones")

            for w_sb, w in ((wq_sb, wq), (wk_sb, wk), (wv_sb, wv)):
                nc.sync.dma_start(
                    out=w_sb, in_=w[:, :].rearrange("(c p) m -> p c m", p=128)
                )
            nc.sync.dma_start(out=wo_sb, in_=wo[:, :])
            for b_sb, bt in ((bq_sb, bq), (bk_sb, bk), (bv_sb, bv)):
                nc.sync.dma_start(out=b_sb, in_=bt[:, :])
            nc.sync.dma_start(out=id_sb, in_=ident[:, :])
            nc.sync.dma_start(out=ones_sb, in_=ones[:, :])

            # ---- persistent activations ----
            qT = persist.tile([128, R], F32R, tag="qT")       # [2*64, rows]
            kT = persist.tile([128, R], F32R, tag="kT")
            ctxT = persist.tile([128, R], F32R, tag="ctxT")
            # V in natural [k-row, dk] layout, augmented with a ones column
            # (col DK) so the ctx matmul also produces the softmax denominator
            v_aug = persist.tile([128, R // 128, HPC, DK + 1], F32R, tag="va")
            nc.sync.dma_start(
                out=v_aug[:, :, :, DK:DK + 1],
                in_=ones[:, :].rearrange("p (a b c) -> p a b c", a=R // 128,
                                         b=HPC, c=1),
            )

            xt_r = xt[:, :].rearrange("(c p) n -> p c n", p=128)

            def do_proj(rc):
                """Project one 512-row chunk into Q^T/K^T and V natural."""
                x_sb = xin.tile([128, D // 128, RCW], F32R, tag="x")
                nc.sync.dma_start(
                    out=x_sb, in_=xt_r[:, :, rc * RCW:(rc + 1) * RCW]
                )
                for w_sb, b_sb, dstT in (
                    (wq_sb, bq_sb, qT),
                    (wk_sb, bk_sb, kT),
                ):
                    ps = psA.tile([128, RCW], F32, tag="s")
                    for c in range(D // 128):
                        nc.tensor.matmul(
                            ps, w_sb[:, c, :], x_sb[:, c, :],
                            start=(c == 0), stop=(c == D // 128 - 1),
                        )
                    nc.vector.tensor_scalar_add(
                        dstT[:, rc * RCW:(rc + 1) * RCW], ps, b_sb
                    )
                ps = psA.tile([128, RCW], F32, tag="s")
                for c in range(D // 128):
                    nc.tensor.matmul(
                        ps, wv_sb[:, c, :], x_sb[:, c, :],
                        start=(c == 0), stop=(c == D // 128 - 1),
                    )
                vt = vtmp.tile([128, RCW], F32, tag="vt")
                nc.vector.tensor_scalar_add(vt, ps, bv_sb)
                # transpose V^T chunk into natural [k-row, dk] tiles
                for blk in range(RCW // 128):
                    ktile = rc * (RCW // 128) + blk
                    for h in range(HPC):
                        tp = psB.tile([128, DK], F32, tag="ce")
                        nc.tensor.transpose(
                            tp,
                            vt[h * DK:(h + 1) * DK, blk * 128:(blk + 1) * 128],
                            id_sb[h * DK:(h + 1) * DK, :],
                        )
                        nc.vector.tensor_copy(v_aug[:, ktile, h, 0:DK], tp)

            def do_attention(b, qc):
                """Attention + output-projection partial for one q chunk."""
                q0 = b * S + qc * QCW
                ps_aug = [
                    psB.tile([DK + 1, QCW], F32, tag="ce", name=f"ps_aug{h}")
                    for h in range(HPC)
                ]
                for kt in range(NKT):
                    k0 = b * S + kt * 128
                    for h in range(HPC):
                        hp = h * DK
                        ps_s = psA.tile([128, QCW], F32, tag="s")
                        for u in range(QCW // 512):
                            nc.tensor.matmul(
                                ps_s[:, u * 512:(u + 1) * 512],
                                kT[hp:hp + DK, k0:k0 + 128],
                                qT[hp:hp + DK, q0 + u * 512:q0 + (u + 1) * 512],
                                start=True, stop=True,
                            )
                        pt = ptp.tile([128, QCW], F32R, tag="pt")
                        nc.scalar.activation(
                            out=pt, in_=ps_s, func=EXP, scale=0.125
                        )
                        for u in range(QCW // 512):
                            nc.tensor.matmul(
                                ps_aug[h][:, u * 512:(u + 1) * 512],
                                v_aug[:, b * NKT + kt, h, :],
                                pt[:, u * 512:(u + 1) * 512],
                                start=(kt == 0), stop=(kt == NKT - 1),
                            )
                for h in range(HPC):
                    # normalize: ctx / ell, where ell sits in row DK of ps_aug
                    cu = recp.tile([DK + 1, QCW], F32R, tag="cu")
                    nc.vector.tensor_copy(cu, ps_aug[h])
                    ps_l = psA.tile([DK, QCW], F32, tag="s")
                    for u in range(QCW // 512):
                        nc.tensor.matmul(
                            ps_l[:, u * 512:(u + 1) * 512],
                            ones_sb[DK:DK + 1, :],
                            cu[DK:DK + 1, u * 512:(u + 1) * 512],
                            start=True, stop=True,
                        )
                    rec = recp.tile([DK, QCW], F32, tag="rec")
                    nc.vector.reciprocal(rec, ps_l)
                    nc.vector.tensor_mul(
                        ctxT[h * DK:(h + 1) * DK, q0:q0 + QCW],
                        cu[0:DK, :], rec,
                    )
                # row-parallel output projection partial for these rows
                for u in range(QCW // 512):
                    for j in range(D // 128):
                        ps_o = psB.tile([128, 512], F32, tag="ce")
                        nc.tensor.matmul(
                            ps_o,
                            wo_sb[:, j * 128:(j + 1) * 128],
                            ctxT[:, q0 + u * 512:q0 + (u + 1) * 512],
                            start=True, stop=True,
                        )
                        ob = ostage.tile([128, 512], F32, tag="ob")
                        nc.vector.tensor_copy(ob, ps_o)
                        nc.sync.dma_start(
                            out=out[j * 128:(j + 1) * 128,
                                    q0 + u * 512:q0 + (u + 1) * 512],
                            in_=ob,
                        )

            # batch 0 projections -> batch 0 attention -> batch 1 ...
            for rc in range(NRC // 2):
                do_proj(rc)
            for qc in range(NQC):
                do_attention(0, qc)
            for rc in range(NRC // 2, NRC):
                do_proj(rc)
            for qc in range(NQC):
                do_attention(1, qc)

    nc.finalize()
    return nc


def _get_nc():
    if "nc" not in _CACHE:
        _CACHE["nc"] = _build_nc()
    return _CACHE["nc"]


def _make_in_maps(x, wq, bq, wk, bk, wv, bv, wo):
    x = np.asarray(x, np.float32)
    xt = np.ascontiguousarray(x.reshape(R, D).T)
    ident = np.zeros((128, DK), np.float32)
    ident[np.arange(128), np.arange(128) % DK] = 1.0
    ones_arr = np.ones((128, DK), np.float32)
    f = lambda a: np.asarray(a, np.float32)
    in_maps = []
    for c in range(NCORES):
        lo, hi = c * PC, (c + 1) * PC
        in_maps.append({
            "xt": xt,
            "wq": np.ascontiguousarray(f(wq)[:, lo:hi]),
            "wk": np.ascontiguousarray(f(wk)[:, lo:hi]),
            "wv": np.ascontiguousarray(f(wv)[:, lo:hi]),
            "wo": np.ascontiguousarray(f(wo)[lo:hi, :]),
            "bq": np.ascontiguousarray(f(bq)[lo:hi]).reshape(PC, 1),
            "bk": np.ascontiguousarray(f(bk)[lo:hi]).reshape(PC, 1),
            "bv": np.ascontiguousarray(f(bv)[lo:hi]).reshape(PC, 1),
            "ident": ident,
            "ones": ones_arr,
        })
    return in_maps


def kernel(x, wq, bq, wk, bk, wv, bv, wo, bo):
    nc = _get_nc()
    in_maps = _make_in_maps(x, wq, bq, wk, bk, wv, bv, wo)
    res = run_bass_kernel_spmd(nc, in_maps, core_ids=list(range(NCORES)))
    acc = np.zeros((D, R), np.float64)
    for c in range(NCORES):
        acc += res.results[c]["out"].astype(np.float64)
    acc += np.asarray(bo, np.float64)[:, None]
    return np.ascontiguousarray(acc.T).astype(np.float32).reshape(B, S, D)
